# revision 1
# baseline (speedup 1.0000x reference)
"""Causal multi-head attention block (QKV proj -> causal MHA -> out proj) on 8 Trainium2
cores.

Sharding: core = b*2 + hh handles batch b (of 4) and head-half hh (8 of 16 heads),
computing attention for its heads over the full sequence, then a partial output
projection over its 512 y-channels for all 2048 tokens. A pairwise ReduceScatter
([0,1],[2,3],...) sums the two partials of each batch and leaves each core with its
token-half of the final output. Host-side work is pure slicing/concatenation.

Matmuls run in float32r (single-pass reduced-precision fp32 on the PE, ~1e-4 rel err);
everything else is fp32.
"""

import numpy as np

import concourse.bass as bass
import concourse.tile as tile
from concourse import bacc, mybir
from concourse.bass_utils import run_bass_kernel_spmd

F32 = mybir.dt.float32
F32R = mybir.dt.float32r
AF = mybir.ActivationFunctionType

B, T, C, H = 4, 2048, 1024, 16
D = C // H          # 64
NHL = H // 2        # 8 local heads per core
NHP = NHL // 2      # 4 local head pairs
FL = NHL * D        # 512 local features
NCC = C // 128      # 8 contraction chunks over C
NTB = T // 128      # 16 token blocks
NTT = T // 512      # 4 token tiles / qtiles
NEG = -1.0e30


def build():
    nc = bacc.Bacc("TRN2", target_bir_lowering=False, num_devices=8)

    xb = nc.dram_tensor("xb", [T, C], F32R, kind="ExternalInput")
    wq = nc.dram_tensor("wq", [C, FL], F32R, kind="ExternalInput")
    wk = nc.dram_tensor("wk", [C, FL], F32R, kind="ExternalInput")
    wv = nc.dram_tensor("wv", [C, FL], F32R, kind="ExternalInput")
    wo = nc.dram_tensor("wo", [FL, C], F32R, kind="ExternalInput")
    bq = nc.dram_tensor("bq", [FL], F32, kind="ExternalInput")
    bk = nc.dram_tensor("bk", [FL], F32, kind="ExternalInput")
    bvb = nc.dram_tensor("bvb", [128, FL], F32, kind="ExternalInput")
    bob = nc.dram_tensor("bob", [128, C], F32, kind="ExternalInput")  # bo/2 broadcast
    ident = nc.dram_tensor("ident", [128, 128], F32R, kind="ExternalInput")
    mask4 = nc.dram_tensor("mask4", [128, 2048], F32, kind="ExternalInput")
    vones = nc.dram_tensor("vones", [128, NHL], F32R, kind="ExternalInput")
    zh = nc.dram_tensor("zh", [T // 2, C], F32, kind="ExternalOutput")

    with tile.TileContext(nc) as tc:
        with (
            tc.tile_pool(name="res", bufs=1) as res,
            tc.tile_pool(name="dram", bufs=1, space="DRAM") as dram,
        ):
            # resident tensors: Q^T, K^T [128, 4hp x 2048tok]; V+ones [128, 16tb x 520]
            qt_sb = res.tile([128, NHP * T], F32R)
            kt_sb = res.tile([128, NHP * T], F32R)
            v_sb = res.tile([128, NTB * (NHL * 65)], F32R)
            zpart = dram.tile([T, C], F32)
            zreds = [
                dram.tile([128, C], F32, name=f"zred{i}") for i in range(8)
            ]

            # ---------------- phase 1: x^T and QKV projections ----------------
            with (
                tc.tile_pool(name="p1", bufs=3) as p1,
                tc.tile_pool(name="p1c", bufs=1) as p1c,
                tc.tile_pool(name="tp_ps", bufs=4, space="PSUM") as tp_ps_pool,
                tc.tile_pool(name="qkv_ps", bufs=3, space="PSUM") as qkv_ps_pool,
            ):
                id_sb = p1c.tile([128, 128], F32R, tag="ident")
                nc.sync.dma_start(id_sb[:], ident[:, :])
                # warm the exp table set during phase 1 (hides ~2.7us ACT_TABLE_LOAD)
                warm = p1c.tile([1, 1], F32, tag="warm")
                nc.scalar.activation(warm[:], id_sb[0:1, 0:1].bitcast(F32), AF.Exp)
                bq_sb = p1c.tile([128, NHP], F32, tag="bq")
                nc.sync.dma_start(bq_sb[:], bq.rearrange("(f p) -> p f", p=128))
                bk_sb = p1c.tile([128, NHP], F32, tag="bk")
                nc.sync.dma_start(bk_sb[:], bk.rearrange("(f p) -> p f", p=128))
                bvb_sb = p1c.tile([128, FL], F32, tag="bvb")
                nc.sync.dma_start(bvb_sb[:], bvb[:, :])
                wv_sb = p1c.tile([128, NCC * FL], F32R, tag="wv")
                nc.sync.dma_start(
                    wv_sb[:].rearrange("p (c f) -> p c f", c=NCC),
                    wv.rearrange("(c p) f -> p c f", p=128),
                )
                # x^T: [128, 8cc x 2048tok]
                xt = p1c.tile([128, NCC * T], F32R, tag="xt")
                for tt in range(NTT):
                    for tb in range(4 * tt, 4 * tt + 4):
                        xnat = p1.tile([128, C], F32R, tag="xnat", name=f"xnat{tb}")
                        nc.sync.dma_start(xnat[:], xb[tb * 128:(tb + 1) * 128, :])
                        for cg in range(NCC // 4):
                            tp_ps = tp_ps_pool.tile([128, 512], F32R, tag="tp",
                                                    name=f"tp{tb}_{cg}")
                            for k in range(4):
                                cc = cg * 4 + k
                                nc.tensor.transpose(
                                    tp_ps[:, k * 128:(k + 1) * 128],
                                    xnat[:, cc * 128:(cc + 1) * 128], id_sb[:]
                                )
                            nc.scalar.activation(
                                xt[:].rearrange("p (c t) -> p c t", c=NCC)[
                                    :, cg * 4:(cg + 1) * 4, tb * 128:(tb + 1) * 128
                                ],
                                tp_ps[:].rearrange("p (k t) -> p k t", k=4),
                                AF.Copy,
                            )
                    # K^T and Q^T columns for this token tile, with bias
                    for w_dram, b_sb, dst, wnm in (
                        (wk, bk_sb, kt_sb, "k"), (wq, bq_sb, qt_sb, "q")
                    ):
                        for fb in range(NHP):
                            w_t = p1.tile([128, NCC * 128], F32R, tag="wqk",
                                          name=f"w{wnm}{tt}_{fb}")
                            nc.sync.dma_start(
                                w_t[:].rearrange("p (c f) -> p c f", c=NCC),
                                w_dram[:, fb * 128:(fb + 1) * 128].rearrange(
                                    "(c p) f -> p c f", p=128
                                ),
                            )
                            ps = qkv_ps_pool.tile([128, 512], F32, tag="qkv",
                                                  name=f"ps{wnm}{tt}_{fb}")
                            for cc in range(NCC):
                                nc.tensor.matmul(
                                    ps[:],
                                    w_t[:, cc * 128:(cc + 1) * 128],
                                    xt[:, cc * T + tt * 512: cc * T + (tt + 1) * 512],
                                    start=(cc == 0),
                                    stop=(cc == NCC - 1),
                                )
                            nc.scalar.activation(
                                dst[:, fb * T + tt * 512: fb * T + (tt + 1) * 512],
                                ps[:],
                                AF.Identity,
                                bias=b_sb[:, fb:fb + 1],
                            )
                    # V rows for this token tile, with bias + ones columns
                    for tb in range(4 * tt, 4 * tt + 4):
                        ps = qkv_ps_pool.tile([128, 512], F32, tag="qkv",
                                              name=f"psv{tb}")
                        for cc in range(NCC):
                            nc.tensor.matmul(
                                ps[:],
                                xt[:, cc * T + tb * 128: cc * T + (tb + 1) * 128],
                                wv_sb[:, cc * FL:(cc + 1) * FL],
                                start=(cc == 0),
                                stop=(cc == NCC - 1),
                            )
                        vslice = v_sb[:, tb * (NHL * 65):(tb + 1) * (NHL * 65)]
                        v3 = vslice.rearrange("p (h c) -> p h c", h=NHL)
                        nc.vector.tensor_add(
                            v3[:, :, 0:D],
                            ps[:].rearrange("p (h d) -> p h d", h=NHL),
                            bvb_sb[:].rearrange("p (h d) -> p h d", h=NHL),
                        )
                        nc.sync.dma_start(v3[:, :, D:D + 1], vones[:, :].unsqueeze(2))

            # ---------------- phase 2+3: attention, out-proj ----------------
            with (
                tc.tile_pool(name="ysb_pool", bufs=1) as ysb_pool,
                tc.tile_pool(name="p2", bufs=6) as p2,
                tc.tile_pool(name="p2c", bufs=1) as p2c,
                tc.tile_pool(name="norm", bufs=3) as norm,
                tc.tile_pool(name="normd", bufs=4, space="DRAM") as normd,
                tc.tile_pool(name="s_ps", bufs=2, space="PSUM") as s_ps_pool,
                tc.tile_pool(name="yu_ps", bufs=2, space="PSUM") as yu_ps_pool,
                tc.tile_pool(name="z_ps", bufs=2, space="PSUM") as z_ps_pool,
            ):
                ysb = ysb_pool.tile([128, NHP * T], F32R)
                m4_sb = p2c.tile([128, 2048], F32, tag="mask")
                nc.sync.dma_start(m4_sb[:], mask4[:, :])

                def attention_qt(qt):
                    for hp in range(NHP):
                        n_kb = 4 * (qt + 1)
                        n_kg = n_kb // 2
                        yus = [
                            yu_ps_pool.tile([65, 512], F32, tag="yu", name=f"yu{qt}_{hp}_{i}")
                            for i in range(2)
                        ]
                        qsl = qt_sb[:, hp * T + qt * 512: hp * T + (qt + 1) * 512]
                        for kg in range(n_kg):
                            # per-kblock causal offset: c = kb - 4*qt in 0..3 on the
                            # diagonal; queries j < c*128 are fully masked -> skip
                            ss = [
                                s_ps_pool.tile([128, 1024], F32, tag="s", name=f"s{qt}_{hp}_{kg}_{i}")
                                for i in range(2)
                            ]
                            j0s = []
                            for c2 in range(2):
                                kb = kg * 2 + c2
                                c = kb - 4 * qt
                                j0s.append(c * 128 if c > 0 else 0)
                            for hi in range(2):
                                for c2 in range(2):
                                    kb = kg * 2 + c2
                                    j0 = j0s[c2]
                                    nc.tensor.matmul(
                                        ss[hi][:, c2 * 512 + j0:(c2 + 1) * 512],
                                        kt_sb[
                                            hi * 64:(hi + 1) * 64,
                                            hp * T + kb * 128: hp * T + (kb + 1) * 128,
                                        ],
                                        qsl[hi * 64:(hi + 1) * 64, j0:],
                                        tile_position=(hi * 64, 0),
                                        start=True,
                                        stop=True,
                                    )
                            for c2 in range(2):
                                kb = kg * 2 + c2
                                c = kb - 4 * qt
                                if 0 <= c <= 3:
                                    # triangular band: only cols [c*128, (c+1)*128)
                                    b0 = c2 * 512 + c * 128
                                    m0 = c * 512 + c * 128
                                    for hi in range(2):
                                        nc.vector.tensor_add(
                                            ss[hi][:, b0:b0 + 128],
                                            ss[hi][:, b0:b0 + 128],
                                            m4_sb[:, m0:m0 + 128],
                                        )
                            for hi in range(2):
                                at = p2.tile([128, 1024], F32R, tag="attn")
                                if j0s[0] >= 256:
                                    # heavily masked pair: exp only valid suffixes
                                    nc.scalar.activation(
                                        at[:, j0s[0]:512], ss[hi][:, j0s[0]:512],
                                        AF.Exp, scale=0.125,
                                    )
                                    nc.scalar.activation(
                                        at[:, 512 + j0s[1]:1024],
                                        ss[hi][:, 512 + j0s[1]:1024],
                                        AF.Exp, scale=0.125,
                                    )
                                else:
                                    nc.scalar.activation(
                                        at[:], ss[hi][:], AF.Exp, scale=0.125
                                    )
                                for c2 in range(2):
                                    kb = kg * 2 + c2
                                    j0 = j0s[c2]
                                    h = 2 * hp + hi
                                    vsl = v_sb[
                                        :,
                                        kb * (NHL * 65) + h * 65:
                                        kb * (NHL * 65) + h * 65 + 65,
                                    ]
                                    nc.tensor.matmul(
                                        yus[hi][:, j0:],
                                        vsl,
                                        at[:, c2 * 512 + j0:(c2 + 1) * 512],
                                        start=(kb == 0),
                                        stop=(kb == n_kb - 1),
                                    )
                        # normalize: y = y_u / rowsum, into ysb feature-major
                        for hi in range(2):
                            rs = norm.tile([65, 512], F32, tag="rs")
                            nc.vector.reciprocal(rs[64:65, :], yus[hi][64:65, :])
                            rs_d = normd.tile([1, 512], F32, tag="rsd",
                                              name=f"rsd{qt}_{hp}_{hi}")
                            nc.sync.dma_start(rs_d[:], rs[64:65, :])
                            bc = norm.tile([64, 512], F32, tag="bc")
                            nc.sync.dma_start(bc[:], rs_d[0:1, :].to_broadcast((64, 512)))
                            ytmp = norm.tile([64, 512], F32R, tag="ytmp")
                            nc.vector.tensor_mul(ytmp[:], yus[hi][0:64, :], bc[:])
                            nc.sync.dma_start(
                                ysb[
                                    hi * 64:(hi + 1) * 64,
                                    hp * T + qt * 512: hp * T + (qt + 1) * 512,
                                ],
                                ytmp[:],
                            )

                # partial out-projection over my 512 channels.
                # zpart rows are chunk-major: [qt0 | qt2 | qt1 | qt3] so each
                # pairwise ReduceScatter chunk is a contiguous 1024-row block.
                # chunk c holds tb c (rank0 tokens) then tb 8+c (rank1 tokens)
                ZROW = {}
                for c in range(8):
                    ZROW[c] = c * 256
                    ZROW[8 + c] = c * 256 + 128

                with (
                    tc.tile_pool(name="p3c", bufs=1) as p3c,
                    tc.tile_pool(name="p3", bufs=3) as p3,
                ):
                    wo_sb = p3c.tile([128, NHP * C], F32R, tag="wo")
                    nc.sync.dma_start(
                        wo_sb[:].rearrange("p (c n) -> p c n", c=NHP),
                        wo.rearrange("(c p) n -> p c n", p=128),
                    )
                    bob_sb = p3c.tile([128, C], F32, tag="bob")
                    nc.sync.dma_start(bob_sb[:], bob[:, :])

                    def out_proj(tbs):
                        for tb in tbs:
                            zrow = ZROW[tb]
                            for ct in range(2):
                                zps = z_ps_pool.tile(
                                    [128, 512], F32, tag="z", name=f"z{tb}_{ct}"
                                )
                                for cc in range(NHP):
                                    nc.tensor.matmul(
                                        zps[:],
                                        ysb[:, cc * T + tb * 128: cc * T + (tb + 1) * 128],
                                        wo_sb[:, cc * C + ct * 512: cc * C + (ct + 1) * 512],
                                        start=(cc == 0),
                                        stop=(cc == NHP - 1),
                                    )
                                z_sb = p3.tile(
                                    [128, 512], F32, tag="zsb", name=f"zsb{tb}_{ct}"
                                )
                                nc.vector.tensor_add(
                                    z_sb[:], zps[:], bob_sb[:, ct * 512:(ct + 1) * 512]
                                )
                                nc.sync.dma_start(
                                    zpart[zrow:zrow + 128, ct * 512:(ct + 1) * 512],
                                    z_sb[:],
                                )

                    def rs_chunk(c):
                        nc.gpsimd.collective_compute(
                            "ReduceScatter",
                            mybir.AluOpType.add,
                            replica_groups=[[0, 1], [2, 3], [4, 5], [6, 7]],
                            ins=[zpart[c * 256:(c + 1) * 256, :].opt()],
                            outs=[zreds[c].opt()],
                        )
                        nc.sync.dma_start(
                            zh[c * 128:(c + 1) * 128, :], zreds[c][:]
                        )

                    attention_qt(0)
                    attention_qt(2)
                    for c in range(4):
                        out_proj([c, 8 + c])
                        rs_chunk(c)
                    attention_qt(1)
                    attention_qt(3)
                    for c in range(4, 8):
                        out_proj([c, 8 + c])
                        rs_chunk(c)

    nc.compile()
    return nc


_NC_CACHE = None


def _get_nc():
    global _NC_CACHE
    if _NC_CACHE is None:
        _NC_CACHE = build()
    return _NC_CACHE


def _in_maps(x, Wqkv, bqkv, Wo, bo):
    x = np.ascontiguousarray(np.asarray(x, dtype=np.float32))
    Wqkv = np.ascontiguousarray(np.asarray(Wqkv, dtype=np.float32))
    bqkv = np.asarray(bqkv, dtype=np.float32)
    Wo = np.ascontiguousarray(np.asarray(Wo, dtype=np.float32))
    bo = np.asarray(bo, dtype=np.float32)

    ident = np.eye(128, dtype=np.float32)
    i_ = np.arange(128, dtype=np.int64)[:, None]
    j_ = np.arange(512, dtype=np.int64)[None, :]
    mask4 = np.concatenate(
        [np.where(i_ + c * 128 > j_, np.float32(NEG), np.float32(0.0)) for c in range(4)],
        axis=1,
    ).astype(np.float32)

    in_maps = []
    for core in range(8):
        b, hh = core // 2, core % 2
        sl = slice(hh * FL, (hh + 1) * FL)
        bv_loc = bqkv[2 * C:][sl]
        in_maps.append({
            "xb": x[b],
            "wq": np.ascontiguousarray(Wqkv[:, 0 * C:1 * C][:, sl]),
            "wk": np.ascontiguousarray(Wqkv[:, 1 * C:2 * C][:, sl]),
            "wv": np.ascontiguousarray(Wqkv[:, 2 * C:3 * C][:, sl]),
            "wo": np.ascontiguousarray(Wo[sl, :]),
            "bq": np.ascontiguousarray(bqkv[0 * C:1 * C][sl]),
            "bk": np.ascontiguousarray(bqkv[1 * C:2 * C][sl]),
            "bvb": np.broadcast_to(bv_loc[None, :], (128, FL)).copy(),
            "bob": np.broadcast_to((bo * 0.5)[None, :], (128, C)).copy(),
            "ident": ident,
            "vones": np.ones((128, NHL), dtype=np.float32),
            "mask4": mask4,
        })

    return in_maps


def _assemble(res):
    out = np.empty((B, T, C), dtype=np.float32)
    for b in range(B):
        out[b, : T // 2] = res.results[2 * b]["zh"]
        out[b, T // 2:] = res.results[2 * b + 1]["zh"]
    return out


def kernel(x, Wqkv, bqkv, Wo, bo):
    in_maps = _in_maps(x, Wqkv, bqkv, Wo, bo)
    res = run_bass_kernel_spmd(_get_nc(), in_maps, core_ids=list(range(8)))
    return _assemble(res)


def run_traced(x, Wqkv, bqkv, Wo, bo, trace_cores=None):
    in_maps = _in_maps(x, Wqkv, bqkv, Wo, bo)
    res = run_bass_kernel_spmd(
        _get_nc(), in_maps, core_ids=list(range(8)), trace=True,
        trace_cores=trace_cores,
    )
    return res



# revision 2
# speedup vs baseline: 1.3758x; 1.3758x over previous
"""Causal multi-head attention block (QKV proj -> causal MHA -> out proj) on 8 Trainium2
cores.

Sharding: core = b*2 + hh handles batch b (of 4) and head-half hh (8 of 16 heads),
computing attention for its heads over the full sequence, then a partial output
projection over its 512 y-channels for all 2048 tokens. A pairwise ReduceScatter
([0,1],[2,3],...) sums the two partials of each batch and leaves each core with its
token-half of the final output, written directly to the output tensor.

Layout/precision: the host pre-transposes x to [C, T] and pre-lays-out all weights as
exact SBUF images in bf16, so the device does no transposes at all. All matmul operands
are bf16 (PSUM accumulation stays fp32); softmax, normalization, and the final output
are fp32. Causal masking multiplies the bf16 probs by a 0/1 triangle (DVE fast mode)
instead of adding -inf to scores.

Schedule: a single software-pipelined stream. For each query tile qt, the attention
inner loop (scores -> exp -> mask-mult -> attnV, per 128-token key block, double-
buffered through PSUM) is interleaved with the QKV projections of tile qt+1 and the
output projections of tile qt-1, keeping the PE busy while the Activation engine
(exp, the co-critical resource) drains.
"""

import numpy as np
import ml_dtypes

import concourse.bass as bass
import concourse.tile as tile
from concourse import bacc, mybir
from concourse.bass_utils import run_bass_kernel_spmd

F32 = mybir.dt.float32
BF16 = mybir.dt.bfloat16
AF = mybir.ActivationFunctionType

B, T, C, H = 4, 2048, 1024, 16
D = C // H          # 64
NHL = H // 2        # 8 local heads per core
NHP = NHL // 2      # 4 local head pairs
FL = NHL * D        # 512 local features
NCC = C // 128      # 8 contraction chunks over C
NTB = T // 128      # 16 token blocks
NTT = T // 512      # 4 token tiles / qtiles
VW = NHL * 65       # v_sb row stride per token block (8 heads x (64 d + 1 ones))


def build():
    nc = bacc.Bacc("TRN2", target_bir_lowering=False, num_devices=8)

    xtd = nc.dram_tensor("xtd", [128, NCC * T], BF16, kind="ExternalInput")
    wq_d = nc.dram_tensor("wq", [128, NCC * FL], BF16, kind="ExternalInput")
    wk_d = nc.dram_tensor("wk", [128, NCC * FL], BF16, kind="ExternalInput")
    wv_d = nc.dram_tensor("wv", [128, NCC * FL], BF16, kind="ExternalInput")
    wo_d = nc.dram_tensor("wo", [128, NHP * C], BF16, kind="ExternalInput")
    bq_d = nc.dram_tensor("bq", [128, NHP], F32, kind="ExternalInput")
    bk_d = nc.dram_tensor("bk", [128, NHP], F32, kind="ExternalInput")
    bvb_d = nc.dram_tensor("bvb", [128, FL], F32, kind="ExternalInput")
    bob_d = nc.dram_tensor("bob", [128, C], F32, kind="ExternalInput")  # bo/2 broadcast
    tri_d = nc.dram_tensor("tri", [128, 128], BF16, kind="ExternalInput")
    zh = nc.dram_tensor("zh", [T // 2, C], F32, kind="ExternalOutput")

    with tile.TileContext(nc) as tc:
        with (
            tc.tile_pool(name="res", bufs=1) as res,
            tc.tile_pool(name="dram", bufs=1, space="DRAM") as dram,
            tc.tile_pool(name="mm_ps", bufs=2, space="PSUM") as mm_ps,
            tc.tile_pool(name="s_ps", bufs=2, space="PSUM") as s_ps,
            tc.tile_pool(name="yu_ps", bufs=2, space="PSUM") as yu_ps,
            tc.tile_pool(name="atp", bufs=4) as atp,
            tc.tile_pool(name="nrm", bufs=4) as nrm,
            tc.tile_pool(name="zp", bufs=3) as zp,
        ):
            wq_sb = res.tile([128, NCC * FL], BF16)
            wk_sb = res.tile([128, NCC * FL], BF16)
            wv_sb = res.tile([128, NCC * FL], BF16)
            wo_sb = res.tile([128, NHP * C], BF16)
            xt = res.tile([128, NCC * T], BF16)
            qt_sb = res.tile([128, NHP * T], BF16)
            kt_sb = res.tile([128, NHP * T], BF16)
            v_sb = res.tile([128, NTB * VW], BF16)
            ysb = res.tile([128, NHP * T], BF16)
            bq_sb = res.tile([128, NHP], F32)
            bk_sb = res.tile([128, NHP], F32)
            bvb_sb = res.tile([128, FL], F32)
            bob_sb = res.tile([128, C], F32)
            tri_sb = res.tile([128, 128], BF16)
            zpart = dram.tile([T, C], F32)

            xt3 = xt[:].rearrange("p (c t) -> p c t", c=NCC)
            xtd3 = xtd.rearrange("p (c t) -> p c t", c=NCC)

            # priority order: first QK weights + x tile 0, then the rest
            nc.sync.dma_start(wq_sb[:], wq_d[:, :])
            nc.sync.dma_start(xt3[:, :, 0:512], xtd3[:, :, 0:512])
            nc.sync.dma_start(wk_sb[:], wk_d[:, :])
            nc.sync.dma_start(wv_sb[:], wv_d[:, :])
            nc.sync.dma_start(bq_sb[:], bq_d[:, :])
            nc.sync.dma_start(bk_sb[:], bk_d[:, :])
            nc.sync.dma_start(bvb_sb[:], bvb_d[:, :])
            nc.sync.dma_start(tri_sb[:], tri_d[:, :])
            nc.sync.dma_start(bob_sb[:], bob_d[:, :])
            nc.sync.dma_start(wo_sb[:], wo_d[:, :])

            # warm the exp table (hides ACT_TABLE_LOAD under the first QKV tile)
            wt = nrm.tile([1, 1], F32, tag="warm")
            nc.gpsimd.memset(wt[:], 0.0)
            nc.scalar.activation(wt[:], wt[:], AF.Exp)

            # constant ones column of V (rowsum trick), written once
            v4 = v_sb[:].rearrange("p (t h c) -> p t h c", t=NTB, h=NHL)
            nc.gpsimd.memset(v4[:, :, :, D:D + 1], 1.0)

            # ---------------- work units ----------------

            def qk_unit(tt, w_sb, b_sb, dst, fb, nm):
                def emit():
                    ps = mm_ps.tile([128, 512], F32, tag="mm",
                                    name=f"{nm}{tt}_{fb}")
                    for cc in range(NCC):
                        nc.tensor.matmul(
                            ps[:],
                            w_sb[:, cc * FL + fb * 128: cc * FL + (fb + 1) * 128],
                            xt[:, cc * T + tt * 512: cc * T + (tt + 1) * 512],
                            start=(cc == 0),
                            stop=(cc == NCC - 1),
                        )
                    nc.vector.tensor_scalar_add(
                        dst[:, fb * T + tt * 512: fb * T + (tt + 1) * 512],
                        ps[:],
                        b_sb[:, fb:fb + 1],
                    )
                return emit

            def v_unit(tb):
                def emit():
                    ps = mm_ps.tile([128, 512], F32, tag="mm", name=f"v{tb}")
                    for cc in range(NCC):
                        nc.tensor.matmul(
                            ps[:],
                            xt[:, cc * T + tb * 128: cc * T + (tb + 1) * 128],
                            wv_sb[:, cc * FL:(cc + 1) * FL],
                            start=(cc == 0),
                            stop=(cc == NCC - 1),
                        )
                    v3 = v_sb[:, tb * VW:(tb + 1) * VW].rearrange(
                        "p (h c) -> p h c", h=NHL)
                    nc.vector.tensor_add(
                        v3[:, :, 0:D],
                        ps[:].rearrange("p (h d) -> p h d", h=NHL),
                        bvb_sb[:].rearrange("p (h d) -> p h d", h=NHL),
                    )
                return emit

            def qkv_units(tt):
                us = []
                if tt > 0:
                    def xdma():
                        nc.sync.dma_start(
                            xt3[:, :, tt * 512:(tt + 1) * 512],
                            xtd3[:, :, tt * 512:(tt + 1) * 512],
                        )
                    us.append(xdma)
                for fb in range(NHP):
                    us.append(qk_unit(tt, wk_sb, bk_sb, kt_sb, fb, "k"))
                    us.append(qk_unit(tt, wq_sb, bq_sb, qt_sb, fb, "q"))
                    us.append(v_unit(4 * tt + fb))
                return us

            # zpart rows are chunk-major so each pairwise ReduceScatter chunk is a
            # contiguous 256-row block: chunk c = [tb c rows | tb 8+c rows].
            ZROW = {}
            for c in range(8):
                ZROW[c] = c * 256
                ZROW[8 + c] = c * 256 + 128

            def op_unit(tb):
                def emit():
                    zrow = ZROW[tb]
                    for ct in range(2):
                        zps = mm_ps.tile([128, 512], F32, tag="mm",
                                         name=f"z{tb}_{ct}")
                        for cc in range(NHP):
                            nc.tensor.matmul(
                                zps[:],
                                ysb[:, cc * T + tb * 128: cc * T + (tb + 1) * 128],
                                wo_sb[:, cc * C + ct * 512: cc * C + (ct + 1) * 512],
                                start=(cc == 0),
                                stop=(cc == NHP - 1),
                            )
                        z_sb = zp.tile([128, 512], F32, tag="z",
                                       name=f"zs{tb}_{ct}")
                        nc.vector.tensor_add(
                            z_sb[:], zps[:], bob_sb[:, ct * 512:(ct + 1) * 512])
                        nc.sync.dma_start(
                            zpart[zrow:zrow + 128, ct * 512:(ct + 1) * 512],
                            z_sb[:],
                        )
                return emit

            def op_units(qt):
                return [op_unit(tb) for tb in range(4 * qt, 4 * qt + 4)]

            def att_closures(qt, hp):
                """Closures for one (qtile, head-pair): per key block kb emit
                scores+exp+mask, with attnV lagging one kb (PSUM double buffer)."""
                n_kb = 4 * (qt + 1)
                st = {"at": {}}

                def start():
                    st["yus"] = [
                        yu_ps.tile([65, 512], F32, tag="yu",
                                   name=f"yu{qt}_{hp}_{i}")
                        for i in range(2)
                    ]

                def scores_kb(kb):
                    c = kb - 4 * qt
                    j0 = c * 128 if c > 0 else 0
                    ss = s_ps.tile([128, 1024], F32, tag="s",
                                   name=f"s{qt}_{hp}_{kb}")
                    for hi in range(2):
                        nc.tensor.matmul(
                            ss[:, hi * 512 + j0:(hi + 1) * 512],
                            kt_sb[hi * 64:(hi + 1) * 64,
                                  hp * T + kb * 128: hp * T + (kb + 1) * 128],
                            qt_sb[hi * 64:(hi + 1) * 64,
                                  hp * T + qt * 512 + j0: hp * T + (qt + 1) * 512],
                            tile_position=(hi * 64, 0),
                            start=True,
                            stop=True,
                        )
                    at = atp.tile([128, 1024], BF16, tag="at",
                                  name=f"at{qt}_{hp}_{kb}")
                    if j0 > 0:
                        ss3 = ss[:].rearrange("p (i x) -> p i x", i=2)
                        at3 = at[:].rearrange("p (i x) -> p i x", i=2)
                        nc.scalar.activation(
                            at3[:, :, j0:512], ss3[:, :, j0:512],
                            AF.Exp, scale=0.125)
                    else:
                        nc.scalar.activation(at[:], ss[:], AF.Exp, scale=0.125)
                    if 0 <= c <= 3:
                        for hi in range(2):
                            b0 = hi * 512 + c * 128
                            nc.vector.tensor_mul(
                                at[:, b0:b0 + 128], at[:, b0:b0 + 128], tri_sb[:])
                    st["at"][kb] = at

                def attnv_kb(kb):
                    c = kb - 4 * qt
                    j0 = c * 128 if c > 0 else 0
                    at = st["at"].pop(kb)
                    for hi in range(2):
                        h = 2 * hp + hi
                        vsl = v_sb[:, kb * VW + h * 65: kb * VW + h * 65 + 65]
                        nc.tensor.matmul(
                            st["yus"][hi][:, j0:],
                            vsl,
                            at[:, hi * 512 + j0:(hi + 1) * 512],
                            start=(kb == 0),
                            stop=(kb == n_kb - 1),
                        )

                def norm():
                    for hi in range(2):
                        rs = nrm.tile([1, 512], F32, tag="rs",
                                      name=f"rs{qt}_{hp}_{hi}")
                        nc.vector.reciprocal(rs[:], st["yus"][hi][64:65, :])
                        bc = nrm.tile([64, 512], F32, tag="bc",
                                      name=f"bc{qt}_{hp}_{hi}")
                        nc.gpsimd.partition_broadcast(bc[:], rs[:])
                        nc.vector.tensor_mul(
                            ysb[hi * 64:(hi + 1) * 64,
                                hp * T + qt * 512: hp * T + (qt + 1) * 512],
                            st["yus"][hi][0:64, :],
                            bc[:],
                        )

                cls = []

                def first():
                    start()
                    scores_kb(0)
                cls.append(first)
                for kb in range(1, n_kb):
                    def mid(kb=kb):
                        scores_kb(kb)
                        attnv_kb(kb - 1)
                    cls.append(mid)

                def last():
                    attnv_kb(n_kb - 1)
                    norm()
                cls.append(last)
                return cls

            # ---------------- schedule ----------------
            for u in qkv_units(0):
                u()
            for qt in range(NTT):
                att = []
                for hp in range(NHP):
                    att += att_closures(qt, hp)
                fillers = []
                if qt < NTT - 1:
                    fillers += qkv_units(qt + 1)
                if qt >= 1:
                    fillers += op_units(qt - 1)
                if fillers:
                    r = len(fillers) / len(att)
                    acc, fi = 0.0, 0
                    for a in att:
                        a()
                        acc += r
                        while acc >= 1.0 and fi < len(fillers):
                            fillers[fi]()
                            fi += 1
                            acc -= 1.0
                    while fi < len(fillers):
                        fillers[fi]()
                        fi += 1
                else:
                    for a in att:
                        a()
            for u in op_units(NTT - 1):
                u()

            # pairwise ReduceScatter straight into the output tensor
            for c in range(8):
                nc.gpsimd.collective_compute(
                    "ReduceScatter",
                    mybir.AluOpType.add,
                    replica_groups=[[0, 1], [2, 3], [4, 5], [6, 7]],
                    ins=[zpart[c * 256:(c + 1) * 256, :].opt()],
                    outs=[zh[c * 128:(c + 1) * 128, :].opt()],
                )

    nc.compile()
    return nc


_NC_CACHE = None


def _get_nc():
    global _NC_CACHE
    if _NC_CACHE is None:
        _NC_CACHE = build()
    return _NC_CACHE


def _in_maps(x, Wqkv, bqkv, Wo, bo):
    bf16 = ml_dtypes.bfloat16
    x = np.asarray(x, dtype=np.float32)
    Wqkv = np.asarray(Wqkv, dtype=np.float32)
    bqkv = np.asarray(bqkv, dtype=np.float32)
    Wo = np.asarray(Wo, dtype=np.float32)
    bo = np.asarray(bo, dtype=np.float32)

    i_ = np.arange(128)[:, None]
    j_ = np.arange(128)[None, :]
    tri = np.where(i_ > j_, 0.0, 1.0).astype(bf16)

    def sbuf_img(w, dt=bf16):
        # [nch*128, f] -> [128, nch*f] SBUF image (partition-major chunks)
        nch = w.shape[0] // 128
        return np.ascontiguousarray(
            w.reshape(nch, 128, -1).transpose(1, 0, 2).reshape(128, -1)
        ).astype(dt)

    in_maps = []
    for core in range(8):
        b, hh = core // 2, core % 2
        sl = slice(hh * FL, (hh + 1) * FL)
        xt_img = sbuf_img(np.ascontiguousarray(x[b].T))
        bv_loc = bqkv[2 * C:3 * C][sl]
        in_maps.append({
            "xtd": xt_img,
            "wq": sbuf_img(np.ascontiguousarray(Wqkv[:, 0 * C:1 * C][:, sl])),
            "wk": sbuf_img(np.ascontiguousarray(Wqkv[:, 1 * C:2 * C][:, sl])),
            "wv": sbuf_img(np.ascontiguousarray(Wqkv[:, 2 * C:3 * C][:, sl])),
            "wo": sbuf_img(np.ascontiguousarray(Wo[sl, :])),
            "bq": np.ascontiguousarray(bqkv[0 * C:1 * C][sl].reshape(NHP, 128).T),
            "bk": np.ascontiguousarray(bqkv[1 * C:2 * C][sl].reshape(NHP, 128).T),
            "bvb": np.broadcast_to(bv_loc[None, :], (128, FL)).copy(),
            "bob": np.broadcast_to((bo * 0.5)[None, :], (128, C)).copy(),
            "tri": tri,
        })
    return in_maps


def _assemble(res):
    out = np.empty((B, T, C), dtype=np.float32)
    for b in range(B):
        out[b, : T // 2] = res.results[2 * b]["zh"]
        out[b, T // 2:] = res.results[2 * b + 1]["zh"]
    return out


def kernel(x, Wqkv, bqkv, Wo, bo):
    in_maps = _in_maps(x, Wqkv, bqkv, Wo, bo)
    res = run_bass_kernel_spmd(_get_nc(), in_maps, core_ids=list(range(8)))
    return _assemble(res)


def run_traced(x, Wqkv, bqkv, Wo, bo, trace_cores=None):
    in_maps = _in_maps(x, Wqkv, bqkv, Wo, bo)
    res = run_bass_kernel_spmd(
        _get_nc(), in_maps, core_ids=list(range(8)), trace=True,
        trace_cores=trace_cores,
    )
    return res
